# revision 20
# baseline (speedup 1.0000x reference)
"""Trainium2 Bass kernel for nn_Half_Graph (GNN message passing block).

Data-parallel over batch: core b processes image b (B=8 across 8 cores).

Per-core layout ("planar G=6"): the 36864-pixel image plane is split into
6 groups of 6144 pixels; a 10-channel tensor occupies 60 SBUF partitions
(partition 10*g + c <-> channel c, pixel group g), padded with 4 zero
rows to 64. Pairs / 20-channel entities use two such 64-row halves at
partitions [0:64] and [64:128] (matmul base partitions must be 0/32/64).
Rows 60..63 / 124..127 are always 0.

All convs are 1x1 so every conv is a matmul over the channel dim with a
block-diagonal (per-group) stationary matrix, BN folded into weights and
bias. Every matmul uses M=128 stationaries (zero cols where inactive) so
PSUM tiles are always fully written. Attention maps are broadcast across
channels with a ones-pattern stationary on the tensor engine (which also
sums the p_att planes for free). The inter-block message sum runs as
identity-matmul accumulation in PSUM. GRU output uses
out = h + u * (c - h).

Host side pre-transposes image planes into the planar layout (cheap, not
part of the measured device time) so every DMA is a plain 2D slice.
"""

import sys

for _p in ("/opt/trn_rl_repo", "/root/.axon_site/_ro/trn_rl_repo"):
    if _p not in sys.path:
        sys.path.insert(0, _p)

import numpy as np

import concourse.bass as bass
import concourse.bacc as bacc
import concourse.mybir as mybir
from concourse.tile import TileContext

F32 = mybir.dt.float32
BF16 = mybir.dt.bfloat16
AL = mybir.AluOpType
AF = mybir.ActivationFunctionType

B = 8
HD = 10
HW = 192 * 192          # 36864 pixels
G = 6                   # pixel groups
GP = HW // G            # 6144 pixels per group
CW = 1024               # chunk width (columns per group per chunk)
NCHUNK = GP // CW       # 6 chunks
EPS = 1e-5
H1 = 64                 # partition offset of half 1

# stationary matrix indices
(S_A12, S_UL, S_SPARE,
 S_DW1A0, S_DW1B0, S_UW1A0, S_UW1B0, S_LW1A0, S_LW1B0,
 S_DW1A6, S_DW1B6, S_UW1A6, S_UW1B6, S_LW1A6, S_LW1B6,
 S_DW2H0, S_DW2H1, S_UW2H0, S_UW2H1, S_LW2H0, S_LW2H1,
 S_I0, S_I3,
 S_GUWG_R, S_GLWG_R, S_GUWG_U, S_GLWG_U, S_GUWC, S_GLWC) = range(29)
NS = 29

# bias vector indices
(BV_D1, BV_U1, BV_L1, BV_Z0, BV_Z1, BV_Z3, BV_R, BV_U, BV_C) = range(9)
NB = 9

# comp block processing order and Z-pair mapping:
#   Z1 = z_c0 (+) z_c4 ; Z2 = z_c1 (+) z_c5 ; Z3 = z_c2 (+) z_c3
BLOCK_ORDER = [0, 4, 1, 5, 2, 3]
BLOCK_ZPAIR = {0: (1, 0), 4: (1, 1), 1: (2, 0), 5: (2, 1), 2: (3, 0), 3: (3, 1)}
# conv2 stationary per (upper, zhalf)
W2_STAT = {(True, 0): S_UW2H0, (True, 1): S_UW2H1,
           (False, 0): S_LW2H0, (False, 1): S_LW2H1}


def _build_nc():
    nc = bacc.Bacc(trn_type="TRN2")

    # image tensors arrive host-pretransposed to padded planar layout:
    # row 10*g + c <-> (channel c, pixel group g); rows 60..63 zero
    xf2 = nc.declare_dram_parameter("xf2", [128, GP], F32, isOutput=False)
    xh = nc.declare_dram_parameter("xh", [128, GP], F32, isOutput=False)
    xhB = nc.declare_dram_parameter("xhB", [128, GP], BF16, isOutput=False)
    xp = nc.declare_dram_parameter("xp", [4, 128, GP], F32, isOutput=False)
    hatt = nc.declare_dram_parameter("hatt", [12, GP], BF16, isOutput=False)
    patt = nc.declare_dram_parameter("patt", [36, GP], BF16, isOutput=False)
    smats = nc.declare_dram_parameter("smats", [NS, 128, 128], BF16, isOutput=False)
    bvecs = nc.declare_dram_parameter("bvecs", [128, NB], F32, isOutput=False)
    out = nc.declare_dram_parameter("out", [2, 60, GP], F32, isOutput=True)

    def csl(t, j):
        return t[:, j * CW:(j + 1) * CW]

    with TileContext(nc) as tc:
        with (
            tc.tile_pool(name="const", bufs=1) as cpool,
            tc.tile_pool(name="xin", bufs=2) as xin,
            tc.tile_pool(name="xin1", bufs=2) as xin1,
            tc.tile_pool(name="attp", bufs=2) as attp,
            tc.tile_pool(name="pmul", bufs=2) as pmul,
            tc.tile_pool(name="cat", bufs=2) as catp,
            tc.tile_pool(name="hmid", bufs=4) as hpool,
            tc.tile_pool(name="zmid", bufs=5) as zpool,
            tc.tile_pool(name="gmid", bufs=2) as gpool,
            tc.tile_pool(name="gmid1", bufs=1) as gpool1,
            tc.tile_pool(name="psum", bufs=4, space="PSUM") as pp,
        ):
            smt = cpool.tile([128, NS * 128], BF16)
            for n in range(NS):
                nc.sync.dma_start(out=smt[:, n * 128:(n + 1) * 128],
                                  in_=smats[n, :, :])
            bv = cpool.tile([128, NB], F32)
            nc.sync.dma_start(out=bv[:, :], in_=bvecs[:, :])

            def stat(i, K, base=0):
                return smt[base:base + K, i * 128:(i + 1) * 128]

            def mm(psum_tile, s_idx, K, rhs_ap, start, stop, base=0):
                # one logical pass = 512-col matmuls covering CW columns
                lhsT = stat(s_idx, K, base)
                for s in range(0, CW, 512):
                    nc.tensor.matmul(
                        psum_tile[0:128, s:s + 512],
                        lhsT,
                        rhs_ap[:, s:s + 512],
                        start=start, stop=stop)

            def bias(k):
                return bv[0:128, k:k + 1]

            for j in range(NCHUNK):
                # ---------------- loads ----------------
                def pair_load(pool, tag, srcpair):
                    t = pool.tile([128, CW], srcpair.dtype, tag=tag, name=tag)
                    nc.sync.dma_start(out=t[:, :],
                                      in_=srcpair[:, j * CW:(j + 1) * CW])
                    return t

                xhd = pair_load(xin, "xhd", xh)
                xhdB = pair_load(xin, "xhdB", xhB)
                xfd = pair_load(xin1, "xfd", xf2)
                xpd = []
                for pr in range(4):
                    t = xin1.tile([128, CW], F32, tag=f"xpd{pr}", name=f"xpd{pr}")
                    rows = 128 if pr < 2 else 64
                    nc.sync.dma_start(out=t[0:rows, :],
                                      in_=xp[pr, 0:rows, j * CW:(j + 1) * CW])
                    xpd.append(t)
                attA = attp.tile([36, CW], BF16, tag="attA")
                nc.sync.dma_start(out=attA[:, :], in_=csl(patt, j))
                att12 = attp.tile([12, CW], BF16, tag="att12")
                nc.sync.dma_start(out=att12[:, :], in_=csl(hatt, j))
                # GRU Wg concat tiles: bottom halves = xh (HBM re-read)
                catWg_u = catp.tile([128, CW], BF16, tag="catWg_u")
                nc.sync.dma_start(out=catWg_u[64:128, :], in_=xhB[0:64, j * CW:(j + 1) * CW])
                catWg_l = catp.tile([128, CW], BF16, tag="catWg_l")
                nc.sync.dma_start(out=catWg_l[64:128, :], in_=xhB[64:128, j * CW:(j + 1) * CW])
                catWc_u = catp.tile([128, CW], BF16, tag="catWc_u")
                catWc_l = catp.tile([128, CW], BF16, tag="catWc_l")

                # ------------- attention broadcast (PE) -------------
                p_ul = pp.tile([128, CW], F32, tag="ps")
                mm(p_ul, S_UL, 36, attA[0:36, :], True, True)
                p_a12 = pp.tile([128, CW], F32, tag="ps")
                mm(p_a12, S_A12, 12, att12[0:12, :], True, True)

                # ------------- premultiplies (DVE) -------------
                xpm = []
                for pr in range(4):
                    rows = 128 if pr < 2 else 64
                    t = pmul.tile([128, CW], BF16, tag=f"xpm{pr}", name=f"xpm{pr}")
                    nc.vector.tensor_tensor(t[0:rows, :], xpd[pr][0:rows, :],
                                            p_ul[0:rows, :], AL.mult)
                    xpm.append(t)
                xfm = pmul.tile([128, CW], BF16, tag="xfm")
                nc.vector.tensor_tensor(xfm[0:128, :], xfd[0:128, :],
                                        p_a12[0:128, :], AL.mult)

                # ------------- decomposition blocks -------------
                p_du = pp.tile([128, CW], F32, tag="ps")
                mm(p_du, S_DW1A0, 60, xfm[0:60, :], True, False)
                mm(p_du, S_DW1B0, 60, xhdB[0:60, :], False, True)
                H_du = hpool.tile([128, CW], BF16, tag="H")
                nc.scalar.activation(H_du[0:128, :], p_du[0:128, :], AF.Relu,
                                     bias=bias(BV_D1))
                p_dl = pp.tile([128, CW], F32, tag="ps")
                mm(p_dl, S_DW1A6, 60, xfm[H1:H1 + 60, :], True, False, base=H1)
                mm(p_dl, S_DW1B6, 60, xhdB[H1:H1 + 60, :], False, True, base=H1)
                H_dl = hpool.tile([128, CW], BF16, tag="H")
                nc.scalar.activation(H_dl[0:128, :], p_dl[0:128, :], AF.Relu,
                                     bias=bias(BV_D1))
                Z0 = pp.tile([128, CW], F32, tag="ps")
                mm(Z0, S_DW2H0, 128, H_du[0:128, :], True, False)
                mm(Z0, S_DW2H1, 128, H_dl[0:128, :], False, True)
                z0t = zpool.tile([128, CW], BF16, tag="zt")
                nc.vector.tensor_scalar(z0t[0:128, :], Z0[0:128, :],
                                        bias(BV_Z0), 0.0, AL.add, AL.max)

                # ------------- composition blocks -------------
                zpsum = {}
                zt = {}
                for i in BLOCK_ORDER:
                    up = i < 4
                    xh_sl = xhdB[0:60, :] if up else xhdB[H1:H1 + 60, :]
                    sa, ab = (S_UW1A0, 0) if up else (S_LW1A6, H1)
                    t = xpm[i] if up else xpm[i - 4]
                    if up:
                        xpm_sl, sb, bb = t[0:60, :], S_UW1B0, 0
                    else:
                        xpm_sl, sb, bb = t[H1:H1 + 60, :], S_LW1B6, H1
                    p_c = pp.tile([128, CW], F32, tag="ps", name=f"pc{i}")
                    mm(p_c, sa, 60, xh_sl, True, False, base=ab)
                    mm(p_c, sb, 60, xpm_sl, False, True, base=bb)
                    H_c = hpool.tile([128, CW], BF16, tag="H", name=f"Hc{i}")
                    nc.scalar.activation(H_c[0:128, :], p_c[0:128, :], AF.Relu,
                                         bias=bias(BV_U1 if up else BV_L1))
                    zi, half = BLOCK_ZPAIR[i]
                    if zi not in zpsum:
                        zpsum[zi] = pp.tile([128, CW], F32, tag="ps", name=f"zp{zi}")
                    mm(zpsum[zi], W2_STAT[(up, half)], 128, H_c[0:128, :],
                       half == 0, half == 1)
                    if half == 1:
                        bz = BV_Z1 if zi in (1, 2) else BV_Z3
                        zt[zi] = zpool.tile([128, CW], BF16, tag="zt", name=f"zt{zi}")
                        nc.vector.tensor_scalar(zt[zi][0:128, :], zpsum[zi][0:128, :],
                                                bias(bz), 0.0, AL.add, AL.max)

                # ------------- message sum (PE identity) -------------
                p_msg = pp.tile([128, CW], F32, tag="ps")
                mm(p_msg, S_I0, 128, z0t[0:128, :], True, False)
                mm(p_msg, S_I0, 128, zt[1][0:128, :], False, False)
                mm(p_msg, S_I0, 128, zt[2][0:128, :], False, False)
                mm(p_msg, S_I3, 128, zt[3][0:128, :], False, True)
                nc.scalar.activation(catWg_u[0:64, :], p_msg[0:64, :], AF.Copy)
                nc.scalar.activation(catWg_l[0:64, :], p_msg[H1:H1 + 64, :], AF.Copy)
                nc.vector.tensor_copy(catWc_u[0:64, :], p_msg[0:64, :])
                nc.vector.tensor_copy(catWc_l[0:64, :], p_msg[H1:H1 + 64, :])

                # ------------- GRU gates -------------
                p_r = pp.tile([128, CW], F32, tag="ps")
                mm(p_r, S_GUWG_R, 128, catWg_u[0:128, :], True, False)
                mm(p_r, S_GLWG_R, 128, catWg_l[0:128, :], False, True)
                p_u = pp.tile([128, CW], F32, tag="ps")
                mm(p_u, S_GUWG_U, 128, catWg_u[0:128, :], True, False)
                mm(p_u, S_GLWG_U, 128, catWg_l[0:128, :], False, True)
                Rt = gpool.tile([128, CW], BF16, tag="Rt")
                nc.scalar.activation(Rt[0:128, :], p_r[0:128, :], AF.Sigmoid,
                                     bias=bias(BV_R))
                Ut = gpool.tile([128, CW], F32, tag="Ut")
                nc.scalar.activation(Ut[0:128, :], p_u[0:128, :], AF.Sigmoid,
                                     bias=bias(BV_U))

                # rh = r * h into Wc concat bottoms
                nc.gpsimd.tensor_tensor(catWc_u[H1:H1 + 64, :], Rt[0:64, :],
                                        xhdB[0:64, :], AL.mult)
                nc.gpsimd.tensor_tensor(catWc_l[H1:H1 + 64, :], Rt[H1:H1 + 64, :],
                                        xhdB[H1:H1 + 64, :], AL.mult)

                p_cc = pp.tile([128, CW], F32, tag="ps")
                mm(p_cc, S_GUWC, 128, catWc_u[0:128, :], True, False)
                mm(p_cc, S_GLWC, 128, catWc_l[0:128, :], False, True)
                Ct = gpool.tile([128, CW], F32, tag="Ct")
                nc.scalar.activation(Ct[0:128, :], p_cc[0:128, :], AF.Tanh,
                                     bias=bias(BV_C))

                # ------------- GRU combine: out = h + u*(c - h) -------------
                Dt = gpool1.tile([128, CW], F32, tag="Dt")
                nc.gpsimd.tensor_tensor(Dt[0:128, :], Ct[0:128, :],
                                        xhd[0:128, :], AL.subtract)
                Et = gpool1.tile([128, CW], F32, tag="Et")
                nc.gpsimd.tensor_tensor(Et[0:128, :], Ut[0:128, :],
                                        Dt[0:128, :], AL.mult)
                outd = gpool.tile([128, CW], F32, tag="outd")
                nc.gpsimd.tensor_tensor(outd[0:128, :], xhd[0:128, :],
                                        Et[0:128, :], AL.add)

                # ------------- store -------------
                nc.sync.dma_start(out=csl(out[0], j), in_=outd[0:60, :])
                nc.sync.dma_start(out=csl(out[1], j), in_=outd[H1:H1 + 60, :])

    nc.compile()
    return nc


def _fold(W, p):
    g, b, m, v = p[0], p[1], p[2], p[3]
    s = g / np.sqrt(v + EPS)
    return (s[:, None] * W).astype(np.float32), (b - m * s).astype(np.float32)


def _build_params(dW1, dbn1, dW2, dbn2, uW1, ubn1, uW2, ubn2,
                  lW1, lbn1, lW2, lbn2, guWg, gubg, guWc, gubc,
                  glWg, glbg, glWc, glbc):
    dW1f, bd1 = _fold(dW1, dbn1)
    dW2f, bd2 = _fold(dW2, dbn2)
    uW1f, bu1 = _fold(uW1, ubn1)
    uW2f, bu2 = _fold(uW2, ubn2)
    lW1f, bl1 = _fold(lW1, lbn1)
    lW2f, bl2 = _fold(lW2, lbn2)

    S = np.zeros((NS, 128, 128), np.float32)
    ci = np.arange(HD)
    # RC[g, c] = row/col index of (group g, channel c) in a 64-row half
    RC = np.stack([10 * g + ci for g in range(G)])  # [6, 10]

    # attention broadcasts: A12 = h_att1(top)/h_att2(bottom);
    # UL = sum p_att1..4 (top) / sum p_att5..6 (bottom)
    for g in range(G):
        S[S_A12, g, RC[g]] = 1.0
        S[S_A12, 6 + g, H1 + RC[g]] = 1.0
        for k in range(4):
            S[S_UL, 6 * k + g, RC[g]] = 1.0
        for k in (4, 5):
            S[S_UL, 6 * k + g, H1 + RC[g]] = 1.0

    def conv1_stat(i0, i6, Wf, in_off):
        # rows (moving ch ci): base variant handles the 64 offset
        # cols: out ch co -> 10g+co ; out ch 10+co -> 64+10g+co
        for g in range(G):
            S[np.ix_([i0], RC[g], RC[g])] = Wf[0:10, in_off:in_off + 10].T[None]
            S[np.ix_([i0], RC[g], H1 + RC[g])] = Wf[10:20, in_off:in_off + 10].T[None]
            S[np.ix_([i6], H1 + RC[g], RC[g])] = Wf[0:10, in_off:in_off + 10].T[None]
            S[np.ix_([i6], H1 + RC[g], H1 + RC[g])] = Wf[10:20, in_off:in_off + 10].T[None]

    conv1_stat(S_DW1A0, S_DW1A6, dW1f, 0)    # xf*att part (concat ch 0..9)
    conv1_stat(S_DW1B0, S_DW1B6, dW1f, 10)   # xh part
    conv1_stat(S_UW1A0, S_UW1A6, uW1f, 0)    # xh part (first in concat)
    conv1_stat(S_UW1B0, S_UW1B6, uW1f, 10)   # xp*att part
    conv1_stat(S_LW1A0, S_LW1A6, lW1f, 0)
    conv1_stat(S_LW1B0, S_LW1B6, lW1f, 10)

    def conv2_stat(ih0, ih1, Wf):
        # moving rows: H planar [0:60]=in ch 0..9, [64:124]=in ch 10..19
        # cols: out ch co -> 10g+co (H0 variant) or 64+10g+co (H1 variant)
        for g in range(G):
            for idx, off in ((ih0, 0), (ih1, H1)):
                S[np.ix_([idx], RC[g], off + RC[g])] = Wf[:, 0:10].T[None]
                S[np.ix_([idx], H1 + RC[g], off + RC[g])] = Wf[:, 10:20].T[None]

    conv2_stat(S_DW2H0, S_DW2H1, dW2f)
    conv2_stat(S_UW2H0, S_UW2H1, uW2f)
    conv2_stat(S_LW2H0, S_LW2H1, lW2f)

    for g in range(G):
        S[S_I0, RC[g], RC[g]] = 1.0
        S[S_I0, H1 + RC[g], H1 + RC[g]] = 1.0
        S[S_I3, RC[g], RC[g]] = 1.0
        S[S_I3, H1 + RC[g], RC[g]] = 1.0

    def gru_stat(idx, W, out_rows, off):
        # concat rows: [0:60]=msg (in ch 0..9), [64:124]=h or rh (in ch 10..19)
        for g in range(G):
            S[np.ix_([idx], RC[g], off + RC[g])] = W[out_rows, 0:10].T[None]
            S[np.ix_([idx], H1 + RC[g], off + RC[g])] = W[out_rows, 10:20].T[None]

    gru_stat(S_GUWG_R, guWg, slice(0, 10), 0)
    gru_stat(S_GLWG_R, glWg, slice(0, 10), H1)
    gru_stat(S_GUWG_U, guWg, slice(10, 20), 0)
    gru_stat(S_GLWG_U, glWg, slice(10, 20), H1)
    gru_stat(S_GUWC, guWc, slice(0, 10), 0)
    gru_stat(S_GLWC, glWc, slice(0, 10), H1)

    bvec = np.zeros((128, NB), np.float32)

    def setb(col, top, bot):
        for g in range(G):
            bvec[RC[g], col] = top
            bvec[H1 + RC[g], col] = bot

    setb(BV_D1, bd1[0:10], bd1[10:20])
    setb(BV_U1, bu1[0:10], bu1[10:20])
    setb(BV_L1, bl1[0:10], bl1[10:20])
    setb(BV_Z0, bd2, bd2)
    setb(BV_Z1, bu2, bl2)
    setb(BV_Z3, bu2, bu2)
    setb(BV_R, gubg[0:10], glbg[0:10])
    setb(BV_U, gubg[10:20], glbg[10:20])
    setb(BV_C, gubc, glbc)

    return S, bvec


_NC_CACHE = None


def _get_nc():
    global _NC_CACHE
    if _NC_CACHE is None:
        _NC_CACHE = _build_nc()
    return _NC_CACHE


def _planar(a):
    # [..., HD, H, W] -> [..., 64, GP] zero-padded planar
    lead = a.shape[:-3]
    a = np.asarray(a, np.float32).reshape(lead + (HD, G, GP))
    a = np.moveaxis(a, -2, -3)          # [..., G, HD, GP]
    a = a.reshape(lead + (60, GP))
    pad = np.zeros(lead + (4, GP), np.float32)
    return np.ascontiguousarray(np.concatenate([a, pad], axis=-2))


def _att_planar(a):
    # [K, H, W] -> [6K, GP]: row 6*k + g
    K = a.shape[0]
    return np.ascontiguousarray(np.asarray(a, np.float32).reshape(K * G, GP))


def _unplanar(a):
    # [..., 60, GP] -> [..., HD, H, W]
    lead = a.shape[:-2]
    a = a.reshape(lead + (G, HD, GP))
    a = np.moveaxis(a, -3, -2)          # [..., HD, G, GP]
    return a.reshape(lead + (HD, 192, 192))


BF_NP = mybir.dt.np(mybir.dt.bfloat16)


def make_in_maps(xf, xh, xp, h_att, p_att, smats, bvecs):
    smatsB = smats.astype(BF_NP)
    in_maps = []
    for b in range(B):
        xhP = _planar(xh[:, b])           # [2, 64, GP]
        xhPair = np.ascontiguousarray(xhP.reshape(128, GP))
        xfP = _planar(xf[b])              # [64, GP]
        xpP = _planar(xp[:, b])           # [6, 64, GP]
        zz = np.zeros((64, GP), np.float32)
        xpPairs = np.ascontiguousarray(np.stack([
            np.concatenate([xpP[0], xpP[4]], axis=0),
            np.concatenate([xpP[1], xpP[5]], axis=0),
            np.concatenate([xpP[2], zz], axis=0),
            np.concatenate([xpP[3], zz], axis=0)]))
        in_maps.append(dict(
            xf2=np.ascontiguousarray(np.concatenate([xfP, xfP], axis=0)),
            xh=xhPair,
            xhB=xhPair.astype(BF_NP),
            xp=xpPairs,
            hatt=_att_planar(h_att[1:3, b, 0]).astype(BF_NP),
            patt=_att_planar(p_att[1:7, b, 0]).astype(BF_NP),
            smats=smatsB,
            bvecs=bvecs,
        ))
    return in_maps


def kernel(xf, xh, xp, h_att, p_att,
           dW1, dbn1, dW2, dbn2,
           uW1, ubn1, uW2, ubn2,
           lW1, lbn1, lW2, lbn2,
           guWg, gubg, guWc, gubc,
           glWg, glbg, glWc, glbc,
           _trace=False):
    from concourse.bass_utils import run_bass_kernel_spmd

    args = [np.asarray(a, dtype=np.float32) for a in
            (dW1, dbn1, dW2, dbn2, uW1, ubn1, uW2, ubn2,
             lW1, lbn1, lW2, lbn2, guWg, gubg, guWc, gubc,
             glWg, glbg, glWc, glbc)]
    smats, bvecs = _build_params(*args)
    in_maps = make_in_maps(np.asarray(xf, np.float32), np.asarray(xh, np.float32),
                           np.asarray(xp, np.float32),
                           np.asarray(h_att, np.float32),
                           np.asarray(p_att, np.float32), smats, bvecs)

    nc = _get_nc()
    res = run_bass_kernel_spmd(nc, in_maps, core_ids=list(range(B)),
                               trace=_trace)
    out = np.empty((2, B, HD, 192, 192), np.float32)
    for b in range(B):
        out[:, b] = _unplanar(res.results[b]["out"])
    if _trace:
        return out, res
    return out
